# revision 69
# baseline (speedup 1.0000x reference)
"""BinaryConv2d on 8 TRN2 NeuronCores — 1-D Winograd F(2,3) along H, fp16.

Problem: x (32,256,56,56) f32, weights (256,256,3,3) f32.
  out = conv2d(x, sign(weights)), NCHW/OIHW, stride 1, VALID -> (32,256,54,54).

Data-parallel: 4 images per core, weights replicated. The 3 kh-taps are
replaced by 4 Winograd row-planes shared by 2 output rows each, cutting
PE work to 2/3 of direct conv (the kw taps stay direct, so rhs slices
are the baseline's proven strided [128, rows, 54] shape):

  input transform (DVE, fp16, per channel plane ct, per row-pair i):
    T1[i] = x[2i+1] + x[2i+2]      T2[i] = x[2i+2] - x[2i+1]
    T0[i] = x[2i]   - x[2i+2]      T3[i] = x[2i+1] - x[2i+3]
  matmul (PE, fp16, per plane p: 6 accumulating matmuls = 3 kw x 2 ct):
    M_p[o, i, j] = sum_{c,kw} U_p[o, c, kw] * T_p[c, i, j+kw]
    with U = G @ sign(w): entries are multiples of 0.5 -> fp16-exact.
  output transform (Scalar copies M1,M2 out of PSUM; DVE combines):
    out[2i]   = M0 + M1 + M2       out[2i+1] = M1 - M2 - M3

27 row-pairs i in blocks of 9; psum free dim 9*54 = 486. The 4 M-planes
of a block live in 4 PSUM banks; tag rings of 2 give full double
buffering (8 banks). Matmul plane order p1,p2,p0,p3 (transforms emitted
in the same order) lets the M1/M2 drains start while p0/p3 stream.

Output path is all-fp16: the scalar engine drains all four PSUM planes
to fp16 SBUF (it sits next to PSUM and is otherwise idle), DVE combines
at fp16 2x rate, and the output DMAs as fp16 (host upcasts) — halving
both the output-transform DVE time and the output HBM traffic.

Scheduling: the startup is HBM-bound (x0 1.6MB + weights). Image 0's
early ot=0 blocks track x-chunk arrivals, with the ot=1 blocks (no new
x rows needed) spliced mid-ramp to absorb chunk-arrival jitter; the
ot=1 weight half is triggered only after image 0's first block so it
loads outside the critical window. x/out DMAs ride separate rings.
Transforms for image n+1 are spliced between the last blocks of image n
— DVE is in-order, so this keeps them from stalling the drain ops that
recycle PSUM banks. The final block is split 7/2; the last sub-block
uses a PSUM-direct f32 drain (p1/p2 copies only — DVE reads ps0/ps3
straight from PSUM, p_order (1,2,3,0)) and triggers its out DMA from
the idle sync queue, so after the final matmul nothing waits on the
scalar queue's copy backlog and the drain chain is just
am -> ob0 -> DMA (~2.9us, at its dependency floor).

Measured rel err ~8.2e-4 (fp16 output rounding included).
PE roofline: 576 matmuls x 486 rows x 0.417ns = 117us vs 175us direct;
measured 139.0us: ~123us PE span + ~4.5us DMA-bound ramp + ~11us tail
(~8.2us of that is the framework's fixed barrier + per-engine
semaphore-reset chain; sem pool size is framework-fixed).
NOTE: the chip P0-downclocks (PE 2.4->2.0GHz, ~+26us) under sustained
back-to-back runs; full-clock 486-row matmuls average ~205-209ns.
"""

import os
import sys

import numpy as np

for _p in ("/opt/trn_rl_repo", "/root/.axon_site/_ro/trn_rl_repo"):
    if os.path.isdir(_p) and _p not in sys.path:
        sys.path.insert(0, _p)

import concourse.bacc as bacc
import concourse.mybir as mybir
from concourse import tile
from concourse.bass_utils import run_bass_kernel_spmd

N_CORES = 8
B, C, H, W = 32, 256, 56, 56
O, KH, KW = 256, 3, 3
OH, OW = H - KH + 1, W - KW + 1  # 54, 54
BPC = B // N_CORES  # images per core
CT = C // 128  # input-channel planes
OT = O // 128  # output-channel tiles
NP = 4  # winograd row planes
NI = OH // 2  # 27 row-pair tiles
IB = 9  # row-pairs per matmul block
NB = NI // IB  # 3 blocks
HWF = H * W
XCHUNKS = (0, 7, 13, 20, 29, 38, 56)
# image-0 chunks all ride the sync ring IN ORDER (the gpsimd ring's DMAs
# process ~2.5us slower, and out-of-order chunk arrival stalls PE). The
# 5-row first chunk lands earliest and starts 6 of span(0,2)'s 8
# transform ops; the p3 pair (needs row 5) rides chunk 1 — p3 is the
# LAST matmul group of the block, so that wait is mostly hidden.
# (A 6-row first chunk measured 1.6us slower, and merging chunks 1-2
# into (5,12) measured 0.5us slower — block 1 then finished early and
# starved on chunk 2.) Chunk 1 = (5,11): span (2,5)'s p1/p2/p0 planes
# need rows <= 10, so block 1 STARTS on chunk 1 alone and only its
# last (p3) group rides chunk 2 — block 1 runs long enough that chunk 2
# lands before block 2 needs it.
XCHUNKS0 = ((0, 5), (5, 11), (11, 21), (21, 31), (31, 43), (43, 56))
# transform spans: span (i0,i1) needs x rows up to 2*(i1-1)+3.
SPANS0 = ((0, 2), (2, 5), (5, 9), (9, 13), (13, 18), (18, NI))  # image 0
SPANS = ((0, 9), (9, 14), (14, NI))  # steady state
# image-0 blocks: the early ot=0 blocks track x-chunk arrivals; the
# ot=1 blocks (whose weights load deferred, outside the critical DMA
# window) are spliced in mid-ramp — they need NO new x rows, so PE has
# queued work to absorb x-chunk arrival jitter (identical code measured
# 139.0..142.4us purely from early-DMA luck with a pure ot-major order).
BLOCKS0 = (
    (0, 0, 2), (0, 2, 3), (0, 5, 4), (0, 9, 4), (1, 0, 9),
    (0, 13, 5), (1, 9, 9), (0, 18, 9), (1, 18, 9),
)
WARMUP_MM = 6  # dummy matmuls lifting the PE HAM clock-gate during load

_NC_CACHE = {}


def _build():
    nc = bacc.Bacc("TRN2", target_bir_lowering=False, debug=False)
    fp16 = mybir.dt.float16
    f32 = mybir.dt.float32
    ADD = mybir.AluOpType.add
    SUB = mybir.AluOpType.subtract
    COPY = mybir.ActivationFunctionType.Copy
    # (plane, row offset a, op, row offset b): T_p[i] = x[2i+a] op x[2i+b];
    # emitted in the matmul plane order so p1/p2 unlock first.
    TFS = ((1, 1, ADD, 2), (2, 2, SUB, 1), (0, 0, SUB, 2), (3, 1, SUB, 3))
    # x is [n, c, H, ct, W] (ct inside H): an H-chunk DMA is then a single
    # contiguous run per partition (rows*CT*W*2B lines) instead of 2 short
    # runs — DMA chunk latency is descriptor-count bound at startup.
    # w is [c, ot, p, kw, ct, o]: a (ot,p) piece is one contiguous
    # 1536B run per partition (vs 3x512B in p-major order).
    x_d = nc.dram_tensor("x", [BPC, 128, H, CT, W], fp16, kind="ExternalInput")
    w_d = nc.dram_tensor("w", [128, OT, NP, KW, CT, 128], fp16, kind="ExternalInput")
    # fp16 output (host upcasts): halves the out DMA bytes and lets the
    # whole output transform run at DVE 2x fp16 rate; adds ~1e-4 rel err.
    out_d = nc.dram_tensor("out", [BPC, O, OH, OW], fp16, kind="ExternalOutput")
    x_ap = x_d.ap()
    w_ap = w_d.ap()
    out_flat = out_d.ap().rearrange("b o h w -> b o (h w)")

    with tile.TileContext(nc) as tc:
        with (
            tc.tile_pool(name="wpool", bufs=1) as wpool,
            tc.tile_pool(name="xpool", bufs=2) as xpool,
            tc.tile_pool(name="tpool", bufs=2) as tpool,
            # bufs=3: the final rapid-fire sub-blocks outpace the output
            # DMAs — with rings of 2 the last ob tile WAR-waits ~0.9us on
            # the DMA of the block two generations back (seen in trace).
            tc.tile_pool(name="opool", bufs=3) as opool,
            tc.tile_pool(name="pspool", bufs=2, space="PSUM") as pspool,
        ):
            def x_load(n):
                xt = xpool.tile([128, H, CT, W], fp16, tag="x")
                if n == 0:
                    for lo, hi in XCHUNKS0:
                        nc.sync.dma_start(xt[:, lo:hi], x_ap[n, :, lo:hi])
                else:
                    for lo, hi in zip(XCHUNKS, XCHUNKS[1:]):
                        nc.sync.dma_start(xt[:, lo:hi], x_ap[n, :, lo:hi])
                return xt

            def new_t():
                return tpool.tile([128, CT, NP, NI, W], fp16, tag="T", name="tt")

            def new_ts():
                return tpool.tile([128, CT, NP, NI, W], fp16, tag="Ts", name="tts")

            def tf_span(tt, xt, i0, i1):
                ni = i1 - i0
                for p, a, op, b in TFS:
                    for ct in range(CT):
                        nc.vector.tensor_tensor(
                            tt[:, ct, p, i0:i1, :],
                            xt[:, a + 2 * i0 : a + 2 * i0 + 2 * ni - 1 : 2, ct, :],
                            xt[:, b + 2 * i0 : b + 2 * i0 + 2 * ni - 1 : 2, ct, :],
                            op,
                        )

            def tf_shift(ts, xt, i0, i1):
                """One-column-left-shifted T copy: kw=1 matmul reads are
                2-byte offset (misaligned, +8ns each) on the plain T; the
                shifted copy serves them at an aligned offset. Only built
                for images 1-3 — DVE has slack there but image 0's
                transform latency is startup-critical."""
                ni = i1 - i0
                for p, a, op, b in TFS:
                    for ct in range(CT):
                        nc.vector.tensor_tensor(
                            ts[:, ct, p, i0:i1, 0 : W - 1],
                            xt[:, a + 2 * i0 : a + 2 * i0 + 2 * ni - 1 : 2, ct, 1:W],
                            xt[:, b + 2 * i0 : b + 2 * i0 + 2 * ni - 1 : 2, ct, 1:W],
                            op,
                        )

            def emit_block(
                tt, ts, n, ot, i0, nrow, p_order=(1, 2, 0, 3), oq=None,
                legacy=False,
            ):
                """nrow row-pairs starting at row-pair i0 -> 2*nrow out rows."""
                ps = {}
                for p in p_order:
                    ps[p] = pspool.tile(
                        [128, nrow, OW], f32, tag=f"ps{p}", name=f"psb{p}"
                    )
                    psf = ps[p][:].rearrange("q r w -> q (r w)")
                    k = 0
                    for kw in range(KW):
                        for ct in range(CT):
                            if kw == 1 and ts is not None:
                                rhs = ts[:, ct, p, i0 : i0 + nrow, 0:OW]
                            else:
                                rhs = tt[:, ct, p, i0 : i0 + nrow, kw : kw + OW]
                            nc.tensor.matmul(
                                psf,
                                w_sb[:, ot, p, kw, ct],
                                rhs,
                                start=(k == 0),
                                stop=(k == KW * CT - 1),
                            )
                            k += 1
                ve = nc.vector
                ob = opool.tile([128, 2 * nrow, OW], fp16, tag="ob")
                if not legacy:
                    # Scalar drains every PSUM plane to fp16 SBUF (it sits
                    # next to PSUM and is otherwise idle); DVE then combines
                    # at fp16 2x rate with no PSUM-source 1x penalty. Copies
                    # are emitted in plane completion order.
                    s = {}
                    for p in p_order:
                        s[p] = opool.tile(
                            [128, nrow, OW], fp16, tag=f"s{p}", name=f"s{p}"
                        )
                        nc.scalar.activation(s[p][:], ps[p][:], COPY)
                    bm = opool.tile([128, nrow, OW], fp16, tag="bm")
                    am = opool.tile([128, nrow, OW], fp16, tag="am")
                    ve.tensor_tensor(bm[:], s[1][:], s[2][:], SUB)  # M1-M2
                    ve.tensor_tensor(am[:], s[0][:], s[1][:], ADD)  # M0+M1
                    ve.tensor_tensor(ob[:, 0::2, :], am[:], s[2][:], ADD)
                    ve.tensor_tensor(ob[:, 1::2, :], bm[:], s[3][:], SUB)
                else:
                    # Tail-block drain: f32 copies of p1/p2 only; DVE reads
                    # ps0/ps3 straight from PSUM. The last blocks' 4-copy
                    # fp16 drains otherwise pile up on the scalar queue and
                    # delay the final out DMA by over a microsecond.
                    s1 = opool.tile([128, nrow, OW], f32, tag="fs1")
                    s2 = opool.tile([128, nrow, OW], f32, tag="fs2")
                    nc.scalar.activation(s1[:], ps[1][:], COPY)
                    nc.scalar.activation(s2[:], ps[2][:], COPY)
                    bm = opool.tile([128, nrow, OW], f32, tag="fbm")
                    am = opool.tile([128, nrow, OW], f32, tag="fam")
                    ve.tensor_tensor(bm[:], s1[:], s2[:], SUB)  # M1-M2
                    ve.tensor_tensor(ob[:, 1::2, :], bm[:], ps[3][:], SUB)
                    ve.tensor_tensor(am[:], ps[0][:], s1[:], ADD)  # M0+M1
                    ve.tensor_tensor(ob[:, 0::2, :], am[:], s2[:], ADD)
                # out DMA stays on the scalar ring: the sync ring carries x
                # loads, whose early chunks must not queue behind output.
                # The final block triggers from the (by then idle) sync
                # queue so it needn't wait behind scalar's drain copies.
                (oq or nc.scalar).dma_start(
                    out_flat[
                        n,
                        ot * 128 : (ot + 1) * 128,
                        2 * i0 * OW : 2 * (i0 + nrow) * OW,
                    ],
                    ob[:],
                )

            # x triggers first so every DMA queue fires at t=0; then the PE
            # warmup (memset on the otherwise-idle vector engine so gpsimd's
            # queue is free for its x-chunk triggers). Warmup rides
            # generation 0 of the ps1 tag ring; real groups rotate on.
            x0 = x_load(0)
            zt = wpool.tile([128, IB * OW], fp16, tag="warm")
            nc.vector.memset(zt[:], 0.0)
            wps = pspool.tile([128, IB * OW], f32, tag="ps1", name="ps_warm")
            for _ in range(WARMUP_MM):
                nc.tensor.matmul(wps[:], zt[:, :128], zt[:], start=True, stop=True)
            # Weights ride the scalar queue. Only the ot=0 half loads up
            # front; the ot=1 triggers are emitted after image 0's first
            # block (they then execute behind its drain copies, well clear
            # of the x0 chunks but in time for the first ot=1 block).
            w_sb = wpool.tile([128, OT, NP, KW, CT, 128], fp16)
            for p in (1, 2, 0, 3):
                nc.scalar.dma_start(w_sb[:, 0, p], w_ap[:, 0, p])
            t_cur = new_t()
            ts_cur = None  # image 0 runs with unaligned kw=1 reads
            for sp in SPANS0:
                tf_span(t_cur, x0, *sp)

            for n in range(BPC):
                if n == 0:
                    blocks = BLOCKS0
                else:
                    blocks = [(ot, ib * IB, IB) for ot in range(OT) for ib in range(NB)]
                if n < BPC - 1:
                    x_next = x_load(n + 1)
                    t_next = new_t()
                    ts_next = new_ts()
                    units = [(tf_span, t_next, sp) for sp in SPANS] + [
                        (tf_shift, ts_next, sp) for sp in SPANS
                    ]
                else:
                    x_next, t_next, ts_next = None, None, None
                    units = []
                    # split the final block 7/2: only the LAST sub-block
                    # needs to be small (its post-MM drain sets the tail);
                    # more small blocks just burn LDWEIGHTS-bound PE time.
                    # (5/2/2 measured +0.5us, 5/4 +0.5us vs this 7/2.)
                    blocks = blocks[:-1] + [(1, 18, 7)]
                nsp = len(blocks) - len(units)
                for j, (ot, i0, nrow) in enumerate(blocks):
                    emit_block(t_cur, ts_cur, n, ot, i0, nrow)
                    if n == 0 and j == 0:
                        for p in (1, 2, 0, 3):
                            nc.scalar.dma_start(w_sb[:, 1, p], w_ap[:, 1, p])
                    if units and j >= nsp:
                        fn, tile_, sp = units[j - nsp]
                        fn(tile_, x_next, *sp)
                if n == BPC - 1:
                    emit_block(
                        t_cur, ts_cur, n, 1, 25, 2, p_order=(1, 2, 3, 0),
                        oq=nc.sync, legacy=True,
                    )
                t_cur, ts_cur = t_next, ts_next
    nc.compile()
    return nc


def get_nc():
    if "nc" not in _NC_CACHE:
        _NC_CACHE["nc"] = _build()
    return _NC_CACHE["nc"]


def prep_inputs(x, weights):
    """Full f32 inputs -> per-core in_maps (fp16, Winograd weights)."""
    x = np.ascontiguousarray(np.asarray(x, dtype=np.float32))
    weights = np.asarray(weights, dtype=np.float32)
    qw = np.sign(weights).astype(np.float32)  # [O, I, KH, KW]
    G = np.array(
        [[1, 0, 0], [0.5, 0.5, 0.5], [0.5, -0.5, 0.5], [0, 0, 1]], np.float32
    )
    U = np.einsum("pk,oikw->poiw", G, qw)  # multiples of 0.5 -> fp16 exact
    U6 = U.reshape(NP, OT, 128, CT, 128, KW)  # [p, ot, o, ct, c, kw]
    wt = np.transpose(U6, (4, 1, 0, 5, 3, 2))  # [c, ot, p, kw, ct, o]
    w_np = np.ascontiguousarray(wt).astype(np.float16)

    x6 = x.reshape(N_CORES, BPC, CT, 128, H, W)
    x6 = np.transpose(x6, (0, 1, 3, 4, 2, 5))  # [core, n, c128, h, ct, w]
    x_np = np.ascontiguousarray(x6).astype(np.float16)
    return [{"x": x_np[i], "w": w_np} for i in range(N_CORES)]


def run_spmd(in_maps, **kwargs):
    nc = get_nc()
    return run_bass_kernel_spmd(nc, in_maps, list(range(N_CORES)), **kwargs)


def kernel(x, weights):
    in_maps = prep_inputs(x, weights)
    res = run_spmd(in_maps)
    out = np.concatenate(
        [np.asarray(res.results[i]["out"], dtype=np.float32) for i in range(N_CORES)],
        axis=0,
    )
    return np.ascontiguousarray(out)

